# revision 2
# baseline (speedup 1.0000x reference)
"""Trainium2 Bass kernel for nn_Blur: depthwise 4x4 separable blur.

Reference semantics: upfirdn2d(x, k2, up=1, down=1, pad=(2,1,2,1)) with a
separable 4-tap kernel k2 = outer(k1, k1), k1 = [1,3,3,1]/4 * sqrt(4) gain
(flip is a no-op: the kernel is symmetric).
out[h,w] = sum_{i,j} k1[i] k1[j] x[h-2+i, w-2+j] with zero padding.

Bottleneck (established by on-HW ablation): PSUM->SBUF evacuation.  Per
plane, 2x512 f32 per partition (pass-1 intermediate + pass-2 result) must
leave PSUM through ScalarE (1 elem/cyc @ 1.2 GHz) and VectorE (1 elem/cyc
@ 0.96 GHz; PSUM has a single read port per engine, no 2x DVE modes, and
TRN2 matmul can only write f32 PSUM).  Everything else has large slack:
DMA in+out measured 613 GB/s/core (= the 614 GB/s spec), the 16-matmul
banded TensorE stream measured 263 ns/plane, vs ~560 ns/plane per copy
engine.  fp16 I/O is therefore optimal: int8 would halve DMA bytes but
not the per-element copy cost (rel err stays ~3.3e-4, budget 2e-2).

Structure per plane: both 1-D passes are block-banded matmuls with the
data stationary (h = 128*b + p packing, 127/3/3/126-column matmuls per
128-block; boundary columns {127,128,129} accumulate taps from both
h-blocks).  ScalarE copies each plane's pass-1 PSUM to SBUF (gates pass
2, latency-critical, plane-granular); VectorE evacuates pass-2 output in
two-plane pairs ([P,2,2,W] tiles, plane -> own bank) to amortize its
~120-cycle per-instruction PSUM access overhead - measured worth ~8.4 us
over per-plane copies.  Re-balancing copies across the engines (deficit
or lagged assignment), merging the boundary matmuls via has_written
accumulation, and pairing the ScalarE side as well were all implemented
and measured slower (strict-FIFO queue head stalls / PSUM depth limits).

HBM layout: the host pre-packs x (and unpacks out) as [p, c, b, w] with
h = 128*b + p, so each per-partition DMA run is g*2*W*2B = 8 KiB
contiguous on both the HBM and SBUF side.

Sharding: pure data parallel - batch dim (8) across the 8 cores.
"""

import numpy as np

import bass_rust
import concourse.bass as bass
import concourse.mybir as mybir
from concourse.tile import TileContext
from concourse.vector_clock import ScopedClock
from concourse.bass_utils import run_bass_kernel_spmd

N_CORES = 8
C, H, W = 256, 256, 256
P = 128
G = 8  # planes per DMA group (4 MiB fp16 per transfer pair)
PAD0 = 2
TAPS = 4
F32 = mybir.dt.float32

_DT = {"f16": mybir.dt.float16, "f32": mybir.dt.float32}
_NP = {"f16": np.float16, "f32": np.float32}


class _TileContextPatched(TileContext):
    """TileContext whose tail drain splits semaphore waits across
    single-wait nops: the bundled walrus rejects >1 sync wait per
    non-EventSemaphore instruction, while stock Tile piles every live
    semaphore's wait onto the one tail Drain."""

    def _drain_and_barrier(self, tick_clock, wait_clock):
        nc = self.nc
        probe = nc.sync.nop(nofuse=True)
        wait_clock.add_sem_waits(
            probe.ins, ScopedClock({None: tick_clock.global_clock})
        )
        si = probe.ins.sync_info
        waits = list(si.on_wait) if si is not None else []
        updates = list(si.on_update) if si is not None else []
        if len(waits) > 1:
            probe.ins.sync_info = bass_rust.SyncInfo(
                on_wait=waits[:1], on_update=updates
            )
            for w in waits[1:]:
                extra = nc.sync.nop(nofuse=True)
                extra.ins.sync_info = bass_rust.SyncInfo(on_wait=[w], on_update=[])
        nc.sync.drain()
        nc.all_engine_barrier()
        assert self.sems is not None
        popped = nc._tile_sem_poison_stack.pop()
        assert popped is self._sem_poison
        nc.clear_and_free_semaphores(list(self.sems.allocated().values()))
        nc.all_engine_barrier()


def _split_multi_waits(nc: bass.Bass) -> bass.Bass:
    """The bundled walrus accepts at most 1 sync wait per instruction (2
    for EventSemaphore).  Tile's wait assignment attaches up to ~3.  Hoist
    the surplus waits onto same-engine nops inserted right before the
    instruction - the engine is in-order, so semantics are unchanged."""
    ctr = 0
    for f in nc.m.functions:
        for b in f.blocks:
            out = []
            for inst in b.instructions:
                si = inst.sync_info
                limit = 2 if isinstance(inst, mybir.InstEventSemaphore) else 1
                if si is not None and len(si.on_wait) > limit:
                    waits = list(si.on_wait)
                    kept, hoist = waits[-limit:], waits[:-limit]
                    for w in hoist:
                        ctr += 1
                        nop = mybir.InstNoOp(
                            name=f"I-waitsplit-{ctr}", engine=inst.engine
                        )
                        nop.sync_info = bass_rust.SyncInfo(
                            on_wait=[w], on_update=[]
                        )
                        nc.register_instruction(nop)
                        out.append(nop)
                    inst.sync_info = bass_rust.SyncInfo(
                        on_wait=kept, on_update=list(si.on_update)
                    )
                out.append(inst)
            b.instructions[:] = out
    return nc


def _filter_bt(k2: np.ndarray, n: int) -> np.ndarray:
    """B.T for the 1-D pass: B[m, k] = k1[k - m + PAD0], zero-padded edges.

    k1 is recovered from the (separable, rank-1) 2-D kernel: k2 =
    outer(k1, k1), so k1 = k2[0, :] / sqrt(k2[0, 0])."""
    k2 = np.asarray(k2, np.float64)
    k1 = k2[0, :] / np.sqrt(k2[0, 0])
    B = np.zeros((n, n), np.float64)
    for m in range(n):
        for i in range(TAPS):
            k = m + i - PAD0
            if 0 <= k < n:
                B[m, k] = k1[i]
    return np.ascontiguousarray(B.T.astype(np.float32))


def build_nc(
    c_planes: int = C,
    g: int = G,
    mode: str = "f16",
    repeat: int = 1,
    io_bufs: int = 4,
    mid_bufs: int = 8,
    ps_bufs: int = 4,
    plag_k: int = 0,
) -> bass.Bass:
    """One core's program: blur c_planes [H, W] planes independently.

    repeat > 1 re-runs the whole sweep (for slope-based device timing)."""
    assert c_planes % g == 0
    mdt = _DT[mode]
    nc = bass.Bass()
    # h = 128*b + p: partition-major so each partition's (c-group, b, w)
    # slab is one long contiguous HBM run.
    x = nc.dram_tensor("x", [P, c_planes, 2, W], mdt, kind="ExternalInput")
    bt = nc.dram_tensor("bt", [H, H], mdt, kind="ExternalInput")
    out = nc.dram_tensor("out", [P, c_planes, 2, W], mdt, kind="ExternalOutput")

    with _TileContextPatched(nc) as tc:
        with (
            tc.tile_pool(name="const", bufs=1) as cpool,
            tc.tile_pool(name="io", bufs=io_bufs) as iopool,
            tc.tile_pool(name="mid", bufs=mid_bufs) as midpool,
            tc.tile_pool(name="ps1p", bufs=ps_bufs, space="PSUM") as ps1pool,
            tc.tile_pool(name="ps2p", bufs=2, space="PSUM") as pspool,
        ):
            bts = []
            for k in range(2):
                t = cpool.tile([P, H], mdt, tag=f"bt{k}")
                nc.sync.dma_start(out=t[:, :], in_=bt[k * P : (k + 1) * P, :])
                bts.append(t)

            # DVE's per-copy PSUM access overhead (~120 cyc) is amortized
            # by pairing pass-2 PSUM tiles: one 1024-elem out copy per two
            # planes.  Pass-1/mid stay plane-granular on ACT (its copy is
            # not the bottleneck).  Every plag_k-th out-pair moves to the
            # otherwise-idle ACT, emitted one plane late so it is ready at
            # ACT's strict-FIFO queue head.
            pair_n, lag = 0, None
            for gi in [i for _ in range(repeat) for i in range(c_planes // g)]:
                xs = iopool.tile([P, g, 2, W], mdt, tag="x")
                nc.sync.dma_start(
                    out=xs[:, :, :, :], in_=x[:, gi * g : (gi + 1) * g, :, :]
                )
                os = iopool.tile([P, g, 2, W], mdt, tag="o")
                for ci in range(g):
                    # pass 1: tT[w, n] = sum_h x[h, w] * BT[h, n], h-block
                    # banded: block 0 fully covers n in [0,127), block 1
                    # fully covers [130,256); columns {127,128,129} take
                    # taps from both blocks (accumulated pair below).
                    ps1 = ps1pool.tile([P, 2, H], F32, tag="ps1")
                    for wb in range(2):
                        ws = slice(wb * P, (wb + 1) * P)
                        nc.tensor.matmul(
                            ps1[:, wb, 0:127],
                            lhsT=xs[:, ci, 0, ws],
                            rhs=bts[0][:, 0:127],
                            start=True,
                            stop=True,
                        )
                        nc.tensor.matmul(
                            ps1[:, wb, 127:130],
                            lhsT=xs[:, ci, 0, ws],
                            rhs=bts[0][:, 127:130],
                            start=True,
                            stop=False,
                        )
                        nc.tensor.matmul(
                            ps1[:, wb, 127:130],
                            lhsT=xs[:, ci, 1, ws],
                            rhs=bts[1][:, 127:130],
                            start=False,
                            stop=True,
                        )
                        nc.tensor.matmul(
                            ps1[:, wb, 130:256],
                            lhsT=xs[:, ci, 1, ws],
                            rhs=bts[1][:, 130:256],
                            start=True,
                            stop=True,
                        )
                    tt = midpool.tile([P, 2, H], mdt, tag="tt")
                    nc.scalar.copy(out=tt[:, :, :], in_=ps1[:, :, :])
                    if lag is not None and ci % 2 == 0:
                        lps2, los, lci = lag
                        nc.scalar.copy(
                            out=los[:, lci : lci + 2, :, :],
                            in_=lps2[:, :, :, :],
                        )
                        lag = None
                    # pass 2: out[h, w'] = sum_w tT[w, h] * BT[w, w'],
                    # same block-banded structure along w.  Two planes share
                    # one [P, 2, 2, W] tile (plane half -> own bank).
                    if ci % 2 == 0:
                        ps2 = pspool.tile([P, 2, 2, W], F32, tag="ps2")
                    pl = ci % 2
                    for nb in range(2):
                        nsl = slice(nb * P, (nb + 1) * P)
                        nc.tensor.matmul(
                            ps2[:, pl, nb, 0:127],
                            lhsT=tt[:, 0, nsl],
                            rhs=bts[0][:, 0:127],
                            start=True,
                            stop=True,
                        )
                        nc.tensor.matmul(
                            ps2[:, pl, nb, 127:130],
                            lhsT=tt[:, 0, nsl],
                            rhs=bts[0][:, 127:130],
                            start=True,
                            stop=False,
                        )
                        nc.tensor.matmul(
                            ps2[:, pl, nb, 127:130],
                            lhsT=tt[:, 1, nsl],
                            rhs=bts[1][:, 127:130],
                            start=False,
                            stop=True,
                        )
                        nc.tensor.matmul(
                            ps2[:, pl, nb, 130:256],
                            lhsT=tt[:, 1, nsl],
                            rhs=bts[1][:, 130:256],
                            start=True,
                            stop=True,
                        )
                    if ci % 2 == 1:
                        if plag_k and pair_n % plag_k == plag_k - 1 and ci + 1 < g:
                            lag = (ps2, os, ci - 1)
                        else:
                            nc.vector.tensor_copy(
                                out=os[:, ci - 1 : ci + 1, :, :],
                                in_=ps2[:, :, :, :],
                            )
                        pair_n += 1
                nc.sync.dma_start(
                    out=out[:, gi * g : (gi + 1) * g, :, :], in_=os[:, :, :, :]
                )
    return _split_multi_waits(nc)


def pack_x(x_core: np.ndarray, mode: str = "f16") -> np.ndarray:
    """[C, H, W] -> [P, C, 2, W] device layout, h = 128*b + p."""
    xc = np.ascontiguousarray(x_core).astype(_NP[mode])
    return np.ascontiguousarray(
        xc.reshape(C, 2, P, W).transpose(2, 0, 1, 3)
    )


def unpack_out(o_dev: np.ndarray) -> np.ndarray:
    """[P, C, 2, W] device layout -> [C, H, W] float32."""
    return np.ascontiguousarray(
        o_dev.transpose(1, 2, 0, 3)
    ).reshape(C, H, W).astype(np.float32)


def kernel(x: np.ndarray, kernel: np.ndarray) -> np.ndarray:
    x = np.asarray(x)
    in_dtype = x.dtype
    assert x.shape == (N_CORES, C, H, W), x.shape
    btm = _filter_bt(np.asarray(kernel, np.float32), H).astype(np.float16)
    in_maps = [{"x": pack_x(x[i]), "bt": btm} for i in range(N_CORES)]
    res = None
    for attempt in range(3):
        try:
            nc = build_nc()
            res = run_bass_kernel_spmd(nc, in_maps, list(range(N_CORES)))
            break
        except Exception:
            # transient NRT/device hiccups have been observed; rebuild + retry
            if attempt == 2:
                raise
    out = np.stack(
        [unpack_out(res.results[i]["out"]) for i in range(N_CORES)], axis=0
    )
    return out.astype(in_dtype, copy=False)

